# revision 1
# baseline (speedup 1.0000x reference)
# DPP attention kernel for Trainium2 (Bass/Tile), data-parallel over batch.
#
# Reference computation (per example, L=512, D=512):
#   q   = x @ Wq.T + bq ; ql = q*q
#   K   = ql @ ql.T ; d = diag(K)
#   det = (d_i+eps)(d_j+eps) - K*K.T          (K symmetric -> K*K.T = K^2)
#   denom = clamp(sum_strict_upper(det), 1e-9)
#   scores = -(det/denom + d*I)/8 + mask ; P = softmax(scores)
#   h = LN(P @ x @ Wd.T + bd + x)
#
# Mapping notes:
#  - 8 NeuronCores, batch 64 -> 8 examples per core, no collectives.
#  - All big GEMMs run in float32r (FP22 multiplies, fp32 accumulate): at
#    free-dim 512 the PE processes 1 row/cycle, same as bf16, vs 4x cost for
#    true fp32. The BIR verifier requires every producer of an f32r matmul
#    input to round its output, so those tiles are declared float32r and are
#    written by ScalarE (which rounds); PE transposes run in plain fp32.
#  - strict-upper sum uses symmetry: det is bitwise symmetric (K and the
#    outer product are), so sum_triu = (sum_all - trace)/2, with
#    trace = sum_i((d_i+eps)^2 - d_i^2) computed from the d row vector.
#  - softmax skips the max-subtraction: scores = c*det (+mask) with
#    c = -1/(8*denom) < 0 and det > 0, so exp() inputs are <= 0.
#  - softmax normalization (1/rowsum) is applied at the final GEMM epilogue
#    (rows of h scale the same way), fused into the residual add.
#  - fast path (attention_mask == 0, which is what setup_inputs produces):
#    scores are then symmetric, so exp(scores) is its own transpose and the
#    P^T operand of the context GEMM is just E — no PE transposes / copies.
#    A generic masked variant is kept and selected at runtime otherwise.
#  - tensor_tensor_reduce crashes TRN2 hardware; the fused det+rowsum pass
#    uses scalar_tensor_tensor with accum_out instead.

import numpy as np

import concourse.bacc as bacc_mod
import concourse.bass as bass
import concourse.mybir as mybir
import concourse.tile as tile
from concourse.bass import ts
from concourse.masks import make_identity

F32 = mybir.dt.float32
F32R = mybir.dt.float32r
AX = mybir.AxisListType
ALU = mybir.AluOpType
ACT = mybir.ActivationFunctionType

N_CORES = 8
B, L, D = 64, 512, 512
BPC = B // N_CORES  # examples per core
P = 128
NL = L // P  # 4 row chunks
ND = D // P  # 4 feature chunks

DET_EPS = 1e-5
DEN_MIN = 1e-9
LN_EPS = 1e-12
NEG_INV8 = -1.0 / 8.0  # -(1/sqrt(head_size)) with head_size 64


def f(ap):
    return ap.bitcast(F32)


def _emit(nc: bass.Bass, use_mask: bool, trivial_affine: bool):
    x = nc.dram_tensor("x", [BPC, L, D], F32, kind="ExternalInput").ap()
    am = nc.dram_tensor("attention_mask", [BPC, L, L], F32, kind="ExternalInput").ap()
    wq = nc.dram_tensor("Wq", [D, D], F32, kind="ExternalInput").ap()
    bq = nc.dram_tensor("bq", [D], F32, kind="ExternalInput").ap()
    wd = nc.dram_tensor("Wd", [D, D], F32, kind="ExternalInput").ap()
    bd = nc.dram_tensor("bd", [D], F32, kind="ExternalInput").ap()
    lnw = nc.dram_tensor("ln_w", [D], F32, kind="ExternalInput").ap()
    lnb = nc.dram_tensor("ln_b", [D], F32, kind="ExternalInput").ap()
    out = nc.dram_tensor("out", [BPC, L, D], F32, kind="ExternalOutput").ap()

    with tile.TileContext(nc) as tc:
        with (
            tc.tile_pool(name="const", bufs=1) as const,
            tc.tile_pool(name="big", bufs=2) as big,
            tc.tile_pool(name="big3", bufs=(2 if use_mask else 3)) as big3,
            tc.tile_pool(name="mid", bufs=(2 if use_mask else 4)) as mid,
            tc.tile_pool(name="small", bufs=(2 if use_mask else 4)) as small,
            tc.tile_pool(name="ps_gemm", bufs=5, space="PSUM") as ps_gemm,
            tc.tile_pool(name="ps_tr", bufs=2, space="PSUM") as ps_tr,
            tc.tile_pool(name="ps_sm", bufs=1, space="PSUM") as ps_sm,
            tc.tile_pool(name="drm", bufs=2, space="DRAM") as drm,
        ):
            # ---- constants / parameters (once) ----
            ident = const.tile([P, P], F32)
            make_identity(nc, ident)
            ones = const.tile([P, P], F32)
            nc.vector.memset(ones, 1.0)

            eps_c = const.tile([P, 1], F32)
            nc.vector.memset(eps_c, DET_EPS)
            ident_r = const.tile([P, P], F32R)
            nc.vector.tensor_copy(out=ident_r, in_=ident)
            ones_r = const.tile([P, 1], F32R)
            nc.vector.tensor_copy(out=ones_r, in_=ones[:, 0:1])
            magic = const.tile([P, NL], mybir.dt.int32)
            nc.vector.memset(magic, 0x5F37642F)

            # transposed weights (float32r): wT[p, dc, e] = W[e, dc*128+p]
            wqT = const.tile([P, ND, D], F32R)
            wdT = const.tile([P, ND, D], F32R)
            for w_ap, wT in ((wq, wqT), (wd, wdT)):
                w_nat = const.tile([P, ND, D], F32, tag="w_nat")
                for ec in range(ND):
                    nc.sync.dma_start(
                        out=w_nat[:, ec, :],
                        in_=w_ap.rearrange("(c p) d -> p c d", p=P)[:, ec, :],
                    )
                for dc in range(ND):
                    ps = ps_tr.tile([P, D], F32, tag="tr")
                    for ec in range(ND):
                        nc.tensor.transpose(
                            ps[:, ts(ec, P)], w_nat[:, ec, ts(dc, P)], ident
                        )
                    nc.scalar.copy(out=wT[:, dc, :], in_=ps)

            bq_col = const.tile([P, ND], F32)
            nc.sync.dma_start(out=bq_col, in_=bq.rearrange("(c p) -> p c", p=P))
            lnw_b = const.tile([P, D], F32)
            nc.sync.dma_start(out=lnw_b, in_=lnw.unsqueeze(0).to_broadcast([P, D]))
            lnb_b = const.tile([P, D], F32)
            nc.sync.dma_start(out=lnb_b, in_=lnb.unsqueeze(0).to_broadcast([P, D]))
            bd_b = const.tile([P, D], F32)
            nc.sync.dma_start(out=bd_b, in_=bd.unsqueeze(0).to_broadcast([P, D]))

            # ---- per-example pipeline, software-pipelined ----
            # phase A: loads .. exp(E);  phase B: ctxT/h GEMMs + LayerNorm.
            # Emission order A(0) A(1) B(0) A(2) B(1) ... keeps the PE's
            # static schedule busy on example b+1 while b's denominator
            # scalar chain (DVE/ACT latency) resolves.

            def emit_a0(b):
                st = {}
                x_sb = big3.tile([P, NL, D], F32R, tag="x_sb")
                st["x_sb"] = x_sb
                for lc in range(NL):
                    nc.sync.dma_start(
                        out=x_sb[:, lc, :],
                        in_=x[b]
                        .rearrange("(c p) d -> p c d", p=P)[:, lc, :]
                        .bitcast(F32R),
                    )
                if use_mask:
                    mask_sb = big.tile([P, NL, L], F32, tag="mask_sb", bufs=2)
                    st["mask_sb"] = mask_sb
                    nc.sync.dma_start(
                        out=mask_sb, in_=am[b].rearrange("(c p) d -> p c d", p=P)
                    )

                # x transposed (float32r): xT[p, dc, l] = x[l, dc*128+p]
                xT = big.tile([P, ND, L], F32R, tag="xT")
                for dc in range(ND):
                    ps = ps_tr.tile([P, L], F32, tag="tr")
                    for lc in range(NL):
                        nc.tensor.transpose(
                            ps[:, ts(lc, P)].bitcast(F32R), x_sb[:, lc, ts(dc, P)],
                            ident_r,
                        )
                    if dc % 2 == 0:
                        nc.scalar.copy(out=xT[:, dc, :], in_=ps)
                    else:
                        nc.vector.tensor_copy(out=xT[:, dc, :], in_=ps)

                st["xT"] = xT
                return st

            def emit_a(b, st):
                x_sb = st["x_sb"]
                xT = st["xT"]
                # qT = Wq @ x.T (chunked over e), then ql.T = (qT + bq)^2
                qlT = big.tile([P, ND, L], F32R, tag="qlT")
                for ec in range(ND):
                    ps = ps_gemm.tile([P, L], F32, tag="gemm")
                    for dc in range(ND):
                        nc.tensor.matmul(
                            ps, wqT[:, dc, ts(ec, P)], xT[:, dc, :],
                            start=(dc == 0), stop=(dc == ND - 1),
                        )
                    nc.scalar.activation(
                        out=qlT[:, ec, :], in_=ps, func=ACT.Square,
                        bias=bq_col[:, ec : ec + 1],
                    )

                # K = qlT.T @ qlT ; Ksq = K^2 ; kdiag = diag(K) = d
                ksq = big.tile([P, NL, L], F32, tag="ksq", bufs=3)
                kdiag = mid.tile([P, NL, P], F32R, tag="kdiag")
                for ic in range(NL):
                    ps = ps_gemm.tile([P, L], F32, tag="gemm")
                    for ec in range(ND):
                        nc.tensor.matmul(
                            ps, qlT[:, ec, ts(ic, P)], qlT[:, ec, :],
                            start=(ec == 0), stop=(ec == ND - 1),
                        )
                    nc.scalar.activation(out=ksq[:, ic, :], in_=ps, func=ACT.Square)
                    nc.vector.tensor_mul(
                        out=kdiag[:, ic, :], in0=ps[:, ts(ic, P)], in1=ident
                    )

                # d row vector via partition reduce on PE (single f32r matmul)
                drow2 = ps_sm.tile([1, L], F32, tag="sm")
                nc.tensor.matmul(
                    drow2[0:1, :], ones_r[:, 0:1], kdiag, start=True, stop=True
                )
                drow_e = small.tile([1, L], F32, tag="drow_e")
                tsum = small.tile([1, 1], F32, tag="tsum")
                nc.scalar.activation(
                    out=drow_e, in_=drow2, func=ACT.Identity, bias=eps_c[0:1, :],
                    accum_out=tsum,
                )
                # broadcast (d+eps) row across partitions; column copy via the
                # same FP22-stored kdiag values keeps det bitwise symmetric
                de_ps = ps_tr.tile([P, L], F32, tag="tr")
                nc.tensor.matmul(
                    de_ps, ones[0:1, :], drow_e[0:1, :], start=True, stop=True
                )
                dcol4 = small.tile([P, NL], F32, tag="dcol4")
                nc.vector.reduce_sum(out=dcol4, in_=f(kdiag), axis=AX.X)
                de_col = small.tile([P, NL], F32, tag="de_col")
                nc.vector.tensor_scalar_add(out=de_col, in0=dcol4, scalar1=DET_EPS)

                # det = (d_i+eps)(d_j+eps) - Ksq, with per-row sums
                det = big.tile([P, NL, L], F32, tag="det")
                det_rs = small.tile([P, NL], F32, tag="det_rs")
                for ic in range(NL):
                    nc.vector.scalar_tensor_tensor(
                        out=det[:, ic, :], in0=de_ps, scalar=de_col[:, ic : ic + 1],
                        in1=ksq[:, ic, :], op0=ALU.mult, op1=ALU.subtract,
                        accum_out=det_rs[:, ic : ic + 1],
                    )

                # denom = max((sum_all - trace)/2, DEN_MIN); c = -1/(8*denom)
                det_rs1 = small.tile([P, 1], F32, tag="det_rs1")
                nc.vector.reduce_sum(out=det_rs1, in_=det_rs, axis=AX.X)
                s_ps = ps_sm.tile([1, 1], F32, tag="sm")
                nc.tensor.matmul(s_ps, ones[:, 0:1], det_rs1, start=True, stop=True)
                s_sb = small.tile([1, 1], F32, tag="s_sb")
                nc.vector.tensor_copy(out=s_sb, in_=s_ps)
                # trace = 2*eps*T0 + L*eps^2 with T0 = tsum - L*eps
                # denom_raw = S/2 - (eps*tsum - 256*eps^2)
                u1 = small.tile([1, 1], F32, tag="u1")
                nc.vector.tensor_scalar(
                    out=u1, in0=tsum, scalar1=DET_EPS,
                    scalar2=256.0 * DET_EPS * DET_EPS,
                    op0=ALU.mult, op1=ALU.subtract,
                )
                den = small.tile([1, 1], F32, tag="den")
                nc.vector.tensor_scalar(
                    out=den, in0=s_sb, scalar1=0.5, scalar2=u1,
                    op0=ALU.mult, op1=ALU.subtract,
                )
                nc.vector.tensor_scalar_max(out=den, in0=den, scalar1=DEN_MIN)
                crcp = small.tile([1, 1], F32, tag="crcp")
                nc.vector.reciprocal(out=crcp, in_=den)
                c_sb = small.tile([1, 1], F32, tag="c_sb")
                nc.vector.tensor_scalar_mul(out=c_sb, in0=crcp, scalar1=NEG_INV8)

                # broadcast c and denom; dd[:, ic] = d_i * denom
                cb_ps = ps_sm.tile([P, 1], F32, tag="sm")
                nc.tensor.matmul(cb_ps, ones[0:1, :], c_sb, start=True, stop=True)
                c_b = small.tile([P, 1], F32, tag="c_b")
                nc.vector.tensor_copy(out=c_b, in_=cb_ps)
                db_ps = ps_sm.tile([P, 1], F32, tag="sm")
                nc.tensor.matmul(db_ps, ones[0:1, :], den, start=True, stop=True)
                den_b = small.tile([P, 1], F32, tag="den_b")
                nc.vector.tensor_copy(out=den_b, in_=db_ps)
                dd = small.tile([P, NL], F32, tag="dd")
                nc.vector.tensor_scalar_mul(out=dd, in0=dcol4, scalar1=den_b)

                # scores = c*(det + denom*d*I) (+mask) ; E = exp(scores)
                e_rs = small.tile([P, NL], F32, tag="e_rs")
                diagm = mid.tile([P, P], F32, tag="diagm")
                e_sb = big.tile([P, NL, L], F32R, tag="e_sb")
                st["e_sb"] = e_sb
                for ic in range(NL):
                    nc.vector.tensor_scalar_mul(
                        out=diagm, in0=ident, scalar1=dd[:, ic : ic + 1]
                    )
                    nc.gpsimd.tensor_add(
                        out=det[:, ic, ts(ic, P)], in0=det[:, ic, ts(ic, P)],
                        in1=diagm,
                    )
                    if use_mask:
                        nc.vector.scalar_tensor_tensor(
                            out=det[:, ic, :], in0=det[:, ic, :],
                            scalar=c_b[:, 0:1], in1=st["mask_sb"][:, ic, :],
                            op0=ALU.mult, op1=ALU.add,
                        )
                        nc.scalar.activation(
                            out=e_sb[:, ic, :], in_=det[:, ic, :], func=ACT.Exp,
                            accum_out=e_rs[:, ic : ic + 1],
                        )
                    else:
                        nc.scalar.activation(
                            out=e_sb[:, ic, :], in_=det[:, ic, :], func=ACT.Exp,
                            scale=c_b[:, 0:1],
                            accum_out=e_rs[:, ic : ic + 1],
                        )
                inv_rs = small.tile([P, NL], F32, tag="inv_rs")
                st["inv_rs"] = inv_rs
                nc.vector.reciprocal(out=inv_rs, in_=e_rs)
                return st

            def emit_b(b, st):
                x_sb = st["x_sb"]
                e_sb = st["e_sb"]
                inv_rs = st["inv_rs"]
                # P^T operand: E^T. Fast path: scores symmetric -> E^T = E.
                if use_mask:
                    pT = big.tile([P, NL, L], F32R, tag="pT", bufs=2)
                    for jc in range(NL):
                        ps = ps_tr.tile([P, L], F32, tag="tr")
                        for lc in range(NL):
                            nc.tensor.transpose(
                                ps[:, ts(lc, P)].bitcast(F32R),
                                e_sb[:, lc, ts(jc, P)], ident_r,
                            )
                        nc.scalar.copy(out=pT[:, jc, :], in_=ps)
                else:
                    pT = e_sb

                # ctxT = x.T @ E.T (unnormalized)
                ctxT = big.tile([P, ND, L], F32R, tag="ctxT")
                for dc in range(ND):
                    ps = ps_gemm.tile([P, L], F32, tag="gemm")
                    for mc in range(NL):
                        nc.tensor.matmul(
                            ps, x_sb[:, mc, ts(dc, P)], pT[:, mc, :],
                            start=(mc == 0), stop=(mc == NL - 1),
                        )
                    if dc % 2 == 0:
                        nc.scalar.copy(out=ctxT[:, dc, :], in_=ps)
                    else:
                        nc.vector.tensor_copy(out=ctxT[:, dc, :], in_=ps)

                # h = ctx @ Wd.T ; h1 = h*inv_rowsum + x + bd ; LayerNorm
                h1 = big3.tile([P, NL, D], F32, tag="h1")
                mv4 = small.tile([P, NL, 2], F32, tag="mv4")
                for lc in range(NL):
                    ps = ps_gemm.tile([P, D], F32, tag="gemm")
                    for dc in range(ND):
                        nc.tensor.matmul(
                            ps, ctxT[:, dc, ts(lc, P)], wdT[:, dc, :],
                            start=(dc == 0), stop=(dc == ND - 1),
                        )
                    nc.vector.scalar_tensor_tensor(
                        out=h1[:, lc, :], in0=ps, scalar=inv_rs[:, lc : lc + 1],
                        in1=f(x_sb[:, lc, :]), op0=ALU.mult, op1=ALU.add,
                    )
                    if not trivial_affine:
                        nc.gpsimd.tensor_add(
                            out=h1[:, lc, :], in0=h1[:, lc, :], in1=bd_b
                        )
                    stats = mid.tile([P, 6], F32, tag="stats")
                    nc.vector.bn_stats(out=stats, in_=h1[:, lc, :])
                    nc.vector.bn_aggr(out=mv4[:, lc, :], in_=stats)
                # rstd = 1/sqrt(var+eps) on DVE (bit-trick + 2 Newton steps);
                # keeps ScalarE inside the exp/square activation-table set.
                I32 = mybir.dt.int32
                ve = small.tile([P, NL], F32, tag="ve")
                nc.vector.tensor_scalar_add(out=ve, in0=mv4[:, :, 1], scalar1=LN_EPS)
                sh = small.tile([P, NL], I32, tag="sh")
                nc.vector.tensor_scalar(
                    out=sh, in0=ve.bitcast(I32), scalar1=1, scalar2=None,
                    op0=ALU.logical_shift_right,
                )
                rstd4 = small.tile([P, NL], F32, tag="rstd4")
                nc.vector.tensor_sub(out=rstd4.bitcast(I32), in0=magic, in1=sh)
                nrt = small.tile([P, NL], F32, tag="nrt")
                for _ in range(2):
                    nc.vector.tensor_mul(out=nrt, in0=rstd4, in1=rstd4)
                    nc.vector.tensor_mul(out=nrt, in0=nrt, in1=ve)
                    nc.vector.tensor_scalar(
                        out=nrt, in0=nrt, scalar1=-0.5, scalar2=1.5,
                        op0=ALU.mult, op1=ALU.add,
                    )
                    nc.vector.tensor_mul(out=rstd4, in0=rstd4, in1=nrt)
                for lc in range(NL):
                    nc.vector.tensor_scalar(
                        out=h1[:, lc, :], in0=h1[:, lc, :],
                        scalar1=mv4[:, lc, 0:1], scalar2=rstd4[:, lc : lc + 1],
                        op0=ALU.subtract, op1=ALU.mult,
                    )
                    if not trivial_affine:
                        nc.gpsimd.tensor_mul(
                            out=h1[:, lc, :], in0=h1[:, lc, :], in1=lnw_b
                        )
                        nc.gpsimd.tensor_add(
                            out=h1[:, lc, :], in0=h1[:, lc, :], in1=lnb_b
                        )
                    nc.sync.dma_start(
                        out=out[b].rearrange("(c p) d -> p c d", p=P)[:, lc, :],
                        in_=h1[:, lc, :],
                    )

            if use_mask:
                # fallback path: serial emission, smaller tile lifetimes
                for b in range(BPC):
                    st = emit_a0(b)
                    emit_a(b, st)
                    emit_b(b, st)
            else:
                sts = {}
                for b in range(BPC):
                    sts[b] = emit_a0(b)
                    if b >= 1:
                        emit_a(b - 1, sts[b - 1])
                    if b >= 2:
                        emit_b(b - 2, sts.pop(b - 2))
                emit_a(BPC - 1, sts[BPC - 1])
                emit_b(BPC - 2, sts.pop(BPC - 2))
                emit_b(BPC - 1, sts.pop(BPC - 1))
    return nc


_NC_CACHE = {}


def _get_nc(use_mask: bool = False, trivial_affine: bool = True):
    key = (use_mask, trivial_affine)
    if key not in _NC_CACHE:
        nc = bacc_mod.Bacc(trn_type="TRN2", target_bir_lowering=False, debug=False)
        _emit(nc, use_mask, trivial_affine)
        nc.compile()
        _NC_CACHE[key] = nc
    return _NC_CACHE[key]


def kernel(**inputs):
    from concourse.bass_utils import run_bass_kernel_spmd

    x = np.ascontiguousarray(inputs["x"], dtype=np.float32)
    am = np.ascontiguousarray(inputs["attention_mask"], dtype=np.float32)
    shared = {
        k: np.ascontiguousarray(inputs[k], dtype=np.float32)
        for k in ("Wq", "bq", "Wd", "bd", "ln_w", "ln_b")
    }
    trivial = (
        not shared["bd"].any()
        and not shared["ln_b"].any()
        and bool((shared["ln_w"] == 1.0).all())
    )
    nc = _get_nc(use_mask=bool(np.any(am)), trivial_affine=trivial)
    in_maps = []
    for c in range(N_CORES):
        sl = slice(c * BPC, (c + 1) * BPC)
        in_maps.append(
            {"x": x[sl], "attention_mask": am[sl], **shared}
        )
    res = run_bass_kernel_spmd(nc, in_maps, core_ids=list(range(N_CORES)))
    return np.concatenate([r_["out"] for r_ in res.results], axis=0)

